# revision 18
# baseline (speedup 1.0000x reference)
"""Cross-graph attention kernel for Trainium2 (8 NeuronCores, SPMD data-parallel over B).

Problem (B=32 graphs, NA=NB=128 nodes, D=128):
    xa = ha @ W1a.T ; xb = hb @ W1b.T                      (per graph)
    scores[n,m] = sum_h relu(xa[n,h] + xb[m,h] + b1[h]) * w2[h]  (+ b2, which
                  cancels in both softmaxes and is dropped)
    mu_a = ha - softmax_m(scores) @ hb
    mu_b = hb - softmax_n(scores).T @ ha

Sharding: data-parallel over B across 8 cores (4 graphs/core), sim_net params
replicated. All pairwise intermediates stay in SBUF/PSUM.

v2 design notes (from ntff trace analysis of the v1 baseline, 87us):
  - Phase 1 (relu tiles feeding the score matmuls) was producer-bound:
    DVE tensor_scalar 162ns + ACT relu 293ns per [128,128] tile -> ~410ns
    per 4-tile group vs 215ns warm PE matmul. Fix: add GpSimd as a third
    producer (build-time greedy load balancing across DVE/GpSimd/ACT).
  - b1 is folded into xb once per graph (xb' = xb + b1), so the per-n bias
    column is just xa[:, n] - no separate xa+b1 tensor needed.
  - prep matmuls run in bf16 (1 col/cycle vs 4 for fp32).
  - prep(g+1) is hoisted before scores(g) on the PE queue, and all of
    phase 2 for graph g (transpose/attention/outputs) is injected into the
    middle of graph g+1's score stream so the PE never stalls at graph
    boundaries (keeps the HAM clock gate at 2.4GHz too).
  - Input DMAs are issued from the GpSimd queue (25ns/descriptor vs 565ns
    on Sync) with the critical graph-0 slices first; bulk tensors overlap
    on the Sync queue. PE warm-up matmuls run during the DMA wait.
"""

import numpy as np
import ml_dtypes

import concourse.bass as bass
import concourse.tile as tile
from concourse import bacc, mybir
from concourse import bass_utils
from concourse.masks import make_identity

F32 = mybir.dt.float32
BF16 = mybir.dt.bfloat16
AF = mybir.ActivationFunctionType
OP = mybir.AluOpType

B, NA, NB, D = 32, 128, 128, 128
NCORES = 8
G = B // NCORES  # graphs per core
NWARM = 28       # PE warm-up matmuls during the input DMA wait

_CACHE = {}


def _build_program():
    nc = bacc.Bacc(
        "TRN2",
        target_bir_lowering=False,
        debug=False,
        enable_asserts=False,
        num_devices=NCORES,
    )

    # --- DRAM I/O (per core) -------------------------------------------------
    w1aT_d = nc.dram_tensor("w1aT", [D, D], BF16, kind="ExternalInput")
    w1bT_d = nc.dram_tensor("w1bT", [D, D], BF16, kind="ExternalInput")
    b1_d = nc.dram_tensor("b1c", [D, 1], F32, kind="ExternalInput")
    # Group-q stationary for the scores matmul: w2s[h, c] = w2[h]*(c%32==0),
    # c in [0,160); lhsT_q = w2s[:, 32-q : 160-q]. One matmul per q contracts
    # FOUR relu tiles (moving [128, 512]) writing score rows {q,q+32,q+64,q+96};
    # with n = q + 32*j, partition p's own score row lands at free block p//32.
    w2s_d = nc.dram_tensor("w2s", [D, 160], BF16, kind="ExternalInput")
    haTb_d = nc.dram_tensor("haTb", [D, G * NA], BF16, kind="ExternalInput")
    hbTb_d = nc.dram_tensor("hbTb", [D, G * NB], BF16, kind="ExternalInput")
    # [h | -1] bf16, g-blocked along free dim: col g*(D+1)+c
    haEb_d = nc.dram_tensor("haEb", [NA, G * (D + 1)], BF16, kind="ExternalInput")
    hbEb_d = nc.dram_tensor("hbEb", [NB, G * (D + 1)], BF16, kind="ExternalInput")
    # fp32 h for the final mu = h + num*(-1/S) add: col g*D+d
    haF_d = nc.dram_tensor("haF", [NA, G * D], F32, kind="ExternalInput")
    hbF_d = nc.dram_tensor("hbF", [NB, G * D], F32, kind="ExternalInput")
    mua_d = nc.dram_tensor("mu_a", [G * NA, D], F32, kind="ExternalOutput")
    mub_d = nc.dram_tensor("mu_b", [G * NB, D], F32, kind="ExternalOutput")

    mua = mua_d.ap().rearrange("(g n) c -> g n c", g=G)
    mub = mub_d.ap().rearrange("(g n) c -> g n c", g=G)

    with tile.TileContext(nc) as tc:
        with (
            tc.tile_pool(name="data", bufs=1) as data,
            tc.tile_pool(name="gg", bufs=2) as gg,
            tc.tile_pool(name="t", bufs=8) as t_pool,
            tc.tile_pool(name="outs", bufs=4) as out_pool,
            tc.tile_pool(name="prep_ps", bufs=2, space="PSUM") as prep_ps,
            tc.tile_pool(name="sc_ps", bufs=2, space="PSUM") as sc_ps,
            tc.tile_pool(name="ab_ps", bufs=2, space="PSUM") as ab_ps,
        ):
            # --- input DMAs: critical path first, all on the cheap gpsimd queue
            # Graph-0 slices land in their own tiles so the first prep matmul
            # never waits on the bulk (g=1..3) transfers.
            w1aT_sb = data.tile([D, D], BF16, tag="w1aT")
            nc.gpsimd.dma_start(out=w1aT_sb, in_=w1aT_d.ap())
            haTb0 = data.tile([D, NA], BF16, tag="haTb0")
            nc.gpsimd.dma_start(out=haTb0, in_=haTb_d.ap()[:, 0:NA])
            w1bT_sb = data.tile([D, D], BF16, tag="w1bT")
            nc.gpsimd.dma_start(out=w1bT_sb, in_=w1bT_d.ap())
            hbTb0 = data.tile([D, NB], BF16, tag="hbTb0")
            nc.gpsimd.dma_start(out=hbTb0, in_=hbTb_d.ap()[:, 0:NB])
            b1_sb = data.tile([D, 1], F32, tag="b1")
            nc.gpsimd.dma_start(out=b1_sb, in_=b1_d.ap())
            w2s_sb = data.tile([D, 160], BF16, tag="w2s")
            nc.gpsimd.dma_start(out=w2s_sb, in_=w2s_d.ap())
            haTbR = data.tile([D, (G - 1) * NA], BF16, tag="haTbR")
            nc.gpsimd.dma_start(out=haTbR, in_=haTb_d.ap()[:, NA:G * NA])
            hbTbR = data.tile([D, (G - 1) * NB], BF16, tag="hbTbR")
            nc.gpsimd.dma_start(out=hbTbR, in_=hbTb_d.ap()[:, NB:G * NB])
            # graph-0 slices of the phase-2 tensors (needed ~mid-stream)
            E1 = D + 1
            haEb0 = data.tile([NA, E1], BF16, tag="haEb0")
            nc.gpsimd.dma_start(out=haEb0, in_=haEb_d.ap()[:, 0:E1])
            hbEb0 = data.tile([NB, E1], BF16, tag="hbEb0")
            nc.gpsimd.dma_start(out=hbEb0, in_=hbEb_d.ap()[:, 0:E1])
            haF0 = data.tile([NA, D], F32, tag="haF0")
            nc.gpsimd.dma_start(out=haF0, in_=haF_d.ap()[:, 0:D])
            hbF0 = data.tile([NB, D], F32, tag="hbF0")
            nc.gpsimd.dma_start(out=hbF0, in_=hbF_d.ap()[:, 0:D])

            # warm-up fodder + identity (gpsimd), then bulk DMAs on sync queue
            warm_sb = data.tile([128, 16], BF16, tag="warm")
            nc.gpsimd.memset(warm_sb, 0.0)
            ident_bf = data.tile([128, 128], BF16, tag="ident")
            make_identity(nc, ident_bf)

            haEbR = data.tile([NA, (G - 1) * E1], BF16, tag="haEbR")
            nc.sync.dma_start(out=haEbR, in_=haEb_d.ap()[:, E1:G * E1])
            hbEbR = data.tile([NB, (G - 1) * E1], BF16, tag="hbEbR")
            nc.sync.dma_start(out=hbEbR, in_=hbEb_d.ap()[:, E1:G * E1])
            haFR = data.tile([NA, (G - 1) * D], F32, tag="haFR")
            nc.sync.dma_start(out=haFR, in_=haF_d.ap()[:, D:G * D])
            hbFR = data.tile([NB, (G - 1) * D], F32, tag="hbFR")
            nc.sync.dma_start(out=hbFR, in_=hbF_d.ap()[:, D:G * D])

            def _slice(g, t0, tR, w):
                return t0 if g == 0 else tR[:, (g - 1) * w:g * w]

            # --- PE warm-up: keep the HAM activity monitor busy while DMAs land
            ps_warm = prep_ps.tile([128, 16], F32, tag="warm", bufs=1)
            for _ in range(NWARM):
                nc.tensor.matmul(ps_warm, lhsT=ident_bf, rhs=warm_sb,
                                 start=True, stop=True)

            # --- build-time greedy load balancer for the relu tiles ----------
            load = {"v": 0.0, "g": 0.0, "a": 0.0}
            COST = {"v": 165.0, "g": 240.0, "a": 295.0}

            def pick():
                e = min(load, key=lambda k: load[k] + COST[k])
                load[e] += COST[e]
                return e

            xa_bf = [None] * G
            xa_f = [None] * G
            xb_bf = [None] * G

            def emit_prep(g):
                # xa_T[h,n] = W1a @ ha^T ; xb'_T[h,m] = W1b @ hb^T + b1 (bf16)
                ps_xa = prep_ps.tile([D, NA], F32, tag="prep")
                nc.tensor.matmul(ps_xa, lhsT=w1aT_sb,
                                 rhs=_slice(g, haTb0, haTbR, NA),
                                 start=True, stop=True)
                ps_xb = prep_ps.tile([D, NB], F32, tag="prep")
                nc.tensor.matmul(ps_xb, lhsT=w1bT_sb,
                                 rhs=_slice(g, hbTb0, hbTbR, NB),
                                 start=True, stop=True)
                return ps_xa, ps_xb

            def emit_conv(g, ps_xa, ps_xb):
                # per-n bias columns (fp32: Bass requires fp32 scalars for add)
                xa_f[g] = gg.tile([D, NA], F32, tag="xaf", name="xaf")
                nc.scalar.copy(out=xa_f[g], in_=ps_xa)
                load["a"] += 200.0
                xb_bf[g] = gg.tile([D, NB], BF16, tag="xb", name="xb")
                nc.vector.tensor_scalar(
                    out=xb_bf[g], in0=ps_xb, scalar1=b1_sb[:, 0:1],
                    scalar2=None, op0=OP.add)
                load["v"] += 260.0

            ps_xa0, ps_xb0 = emit_prep(0)
            emit_conv(0, ps_xa0, ps_xb0)
            ps_xa1, ps_xb1 = emit_prep(1)
            emit_conv(1, ps_xa1, ps_xb1)

            e_sb = [None] * G
            et_sb = [None] * G
            ps_sc = [None] * G
            ps_tr = [None] * G
            ps_a = [None] * G
            ps_b = [None] * G
            ra = [None] * G
            rb = [None] * G

            def emit_exp(g):
                # E[n, m] = exp(scores): partition range [32u, 32u+32) holds
                # its own scores at free block u. (softmax is shift-invariant;
                # scores are O(+-10) so no max subtraction.)
                e_sb[g] = gg.tile([NA, NB], BF16, tag="e", name="e")
                for u in range(4):
                    nc.scalar.activation(
                        out=e_sb[g][32 * u:32 * (u + 1), :],
                        in_=ps_sc[g][32 * u:32 * (u + 1), u * NB:(u + 1) * NB],
                        func=AF.Exp)
                load["a"] += 4 * 200.0

            def emit_tr(g):
                ps_tr[g] = ab_ps.tile([NB, NA], BF16, tag="tr", name="tr", bufs=1)
                nc.tensor.transpose(ps_tr[g], e_sb[g], ident_bf)

            def emit_attn_b(g):
                # num_b[m, 0:128], -S_ba[m] at col 128
                ps_b[g] = ab_ps.tile([NB, E1], F32, tag="ab", name="psb")
                nc.tensor.matmul(
                    ps_b[g], lhsT=e_sb[g],
                    rhs=_slice(g, haEb0, haEbR, E1), start=True, stop=True)

            def emit_etcopy(g):
                et_sb[g] = gg.tile([NB, NA], BF16, tag="et", name="et")
                nc.vector.tensor_copy(out=et_sb[g], in_=ps_tr[g])
                load["v"] += 190.0

            def emit_attn_a(g):
                ps_a[g] = ab_ps.tile([NA, E1], F32, tag="ab", name="psa")
                nc.tensor.matmul(
                    ps_a[g], lhsT=et_sb[g],
                    rhs=_slice(g, hbEb0, hbEbR, E1), start=True, stop=True)

            def emit_recip(g):
                ra[g] = out_pool.tile([NA, 1], F32, tag="r", name="ra")
                nc.vector.reciprocal(out=ra[g], in_=ps_a[g][:, D:D + 1])
                rb[g] = out_pool.tile([NB, 1], F32, tag="r", name="rb")
                nc.vector.reciprocal(out=rb[g], in_=ps_b[g][:, D:D + 1])
                load["v"] += 330.0

            def emit_stt(g):
                # mu = h + num * (-1/S)
                outa = out_pool.tile([NA, D], F32, tag="oa")
                nc.vector.scalar_tensor_tensor(
                    out=outa, in0=ps_a[g][:, 0:D], scalar=ra[g][:, 0:1],
                    in1=_slice(g, haF0, haFR, D), op0=OP.mult, op1=OP.add)
                load["v"] += 350.0
                outb = out_pool.tile([NB, D], F32, tag="ob")
                nc.vector.scalar_tensor_tensor(
                    out=outb, in0=ps_b[g][:, 0:D], scalar=rb[g][:, 0:1],
                    in1=_slice(g, hbF0, hbFR, D), op0=OP.mult, op1=OP.add)
                load["v"] += 350.0
                return outa, outb

            def emit_out(g, outa, outb):
                nc.gpsimd.dma_start(out=mua[g], in_=outa)
                nc.gpsimd.dma_start(out=mub[g], in_=outb)

            pend = [None]  # phase-2 closure state for the previous graph
            for g in range(G):
                prev = g - 1
                for q in range(32):
                    # inject previous graph's phase 2 into this score stream
                    if prev >= 0:
                        if q == 5:
                            emit_tr(prev)
                            emit_attn_b(prev)
                        elif q == 7:
                            emit_etcopy(prev)
                        elif q == 9:
                            emit_attn_a(prev)
                        elif q == 11:
                            emit_recip(prev)
                        elif q == 12:
                            pend[0] = emit_stt(prev)
                        elif q == 14:
                            emit_out(prev, *pend[0])
                    if q == 20 and g + 1 < G and g >= 1:
                        ps_xa_n, ps_xb_n = emit_prep(g + 1)
                        emit_conv(g + 1, ps_xa_n, ps_xb_n)

                    # 4 relu tiles t_n = relu(xb' + xa_n), n = q + 32j, then
                    # one accumulating score matmul over the group-q stationary
                    if q == 0:
                        ps_sc[g] = sc_ps.tile([NA, 4 * NB], F32, tag="sc", name="sc")
                    t4 = t_pool.tile([D, 4 * NB], BF16, tag="t")
                    for j in range(4):
                        n = q + 32 * j
                        ts = t4[:, j * NB:(j + 1) * NB]
                        eng = pick()
                        if eng == "a":
                            nc.scalar.activation(
                                out=ts, in_=xb_bf[g], func=AF.Relu,
                                bias=xa_f[g][:, n:n + 1], scale=1.0)
                        elif eng == "v":
                            nc.vector.tensor_scalar(
                                out=ts, in0=xb_bf[g],
                                scalar1=xa_f[g][:, n:n + 1], scalar2=0.0,
                                op0=OP.add, op1=OP.max)
                        else:
                            nc.gpsimd.tensor_scalar(
                                out=ts, in0=xb_bf[g],
                                scalar1=xa_f[g][:, n:n + 1], scalar2=0.0,
                                op0=OP.add, op1=OP.max)
                    nc.tensor.matmul(
                        ps_sc[g], lhsT=w2s_sb[:, 32 - q:160 - q], rhs=t4,
                        start=(q == 0), stop=(q == 31))
                emit_exp(g)

            # drain phase 2 of the last graph
            gl = G - 1
            emit_tr(gl)
            emit_attn_b(gl)
            emit_etcopy(gl)
            emit_attn_a(gl)
            emit_recip(gl)
            oa, ob = emit_stt(gl)
            emit_out(gl, oa, ob)

    nc.compile()
    return nc


def _get_program():
    if "nc" not in _CACHE:
        _CACHE["nc"] = _build_program()
    return _CACHE["nc"]


def _prep_in_maps(h_a, h_b, W1, b1, W2):
    h_a = np.asarray(h_a, dtype=np.float32)
    h_b = np.asarray(h_b, dtype=np.float32)
    W1 = np.asarray(W1, dtype=np.float32)
    b1 = np.asarray(b1, dtype=np.float32)
    W2 = np.asarray(W2, dtype=np.float32)
    bf = ml_dtypes.bfloat16

    # W1a[h, d] = W1[h, d], W1b[h, d] = W1[h, D + d]; lhsT wants [d, h].
    w1aT = np.ascontiguousarray(W1[:, :D].T).astype(bf)
    w1bT = np.ascontiguousarray(W1[:, D:].T).astype(bf)
    b1c = np.ascontiguousarray(b1.reshape(D, 1))
    w2bf = W2[0].astype(bf).astype(np.float32)
    comb = (np.arange(160) % 32 == 0).astype(np.float32)
    w2s = np.ascontiguousarray(w2bf[:, None] * comb[None, :]).astype(bf)

    in_maps = []
    for c in range(NCORES):
        ha = h_a[c * G * NA:(c + 1) * G * NA].reshape(G, NA, D)
        hb = h_b[c * G * NB:(c + 1) * G * NB].reshape(G, NB, D)
        neg = np.full((G, NA, 1), -1.0, dtype=np.float32)
        haE = np.concatenate([ha, neg], axis=2)  # [G, NA, D+1]
        hbE = np.concatenate([hb, neg], axis=2)
        in_maps.append({
            "w1aT": w1aT, "w1bT": w1bT, "b1c": b1c, "w2s": w2s,
            "haTb": np.ascontiguousarray(
                ha.transpose(2, 0, 1).reshape(D, G * NA)).astype(bf),
            "hbTb": np.ascontiguousarray(
                hb.transpose(2, 0, 1).reshape(D, G * NB)).astype(bf),
            "haEb": np.ascontiguousarray(
                haE.transpose(1, 0, 2).reshape(NA, G * (D + 1))).astype(bf),
            "hbEb": np.ascontiguousarray(
                hbE.transpose(1, 0, 2).reshape(NB, G * (D + 1))).astype(bf),
            "haF": np.ascontiguousarray(ha.transpose(1, 0, 2).reshape(NA, G * D)),
            "hbF": np.ascontiguousarray(hb.transpose(1, 0, 2).reshape(NB, G * D)),
        })
    return in_maps


def run(h_a, h_b, W1, b1, W2, trace=False, **run_kwargs):
    nc = _get_program()
    in_maps = _prep_in_maps(h_a, h_b, W1, b1, W2)
    res = bass_utils.run_bass_kernel_spmd(
        nc, in_maps, core_ids=list(range(NCORES)), trace=trace, **run_kwargs
    )
    mu_a = np.concatenate([r["mu_a"] for r in res.results], axis=0)
    mu_b = np.concatenate([r["mu_b"] for r in res.results], axis=0)
    return (mu_a, mu_b), res


def kernel(h_a, batch_a, h_b, batch_b, W1, b1, W2, b2):
    # batch_a/batch_b encode the (equal-sized, sorted) graph partition that the
    # dense [B, n, D] view already assumes; b2 shifts scores uniformly and
    # cancels in both softmaxes.
    (mu_a, mu_b), _ = run(h_a, h_b, W1, b1, W2, trace=False)
    return mu_a, mu_b


# revision 20
# speedup vs baseline: 4.5662x; 4.5662x over previous
"""Cross-graph attention kernel for Trainium2 (8 NeuronCores, SPMD data-parallel over B).

Problem (B=32 graphs, NA=NB=128 nodes, D=128):
    xa = ha @ W1a.T ; xb = hb @ W1b.T                      (per graph)
    scores[n,m] = sum_h relu(xa[n,h] + xb[m,h] + b1[h]) * w2[h]  (+ b2, which
                  cancels in both softmaxes and is dropped)
    mu_a = ha - softmax_m(scores) @ hb
    mu_b = hb - softmax_n(scores).T @ ha

Sharding: data-parallel over B across 8 cores (4 graphs/core), sim_net params
replicated. All pairwise intermediates stay in SBUF/PSUM.

v2 design notes (from ntff trace analysis of the v1 baseline, 87us):
  - Phase 1 (relu tiles feeding the score matmuls) was producer-bound:
    DVE tensor_scalar 162ns + ACT relu 293ns per [128,128] tile -> ~410ns
    per 4-tile group vs 215ns warm PE matmul. Fix: add GpSimd as a third
    producer (build-time greedy load balancing across DVE/GpSimd/ACT).
  - b1 is folded into xb once per graph (xb' = xb + b1), so the per-n bias
    column is just xa[:, n] - no separate xa+b1 tensor needed.
  - prep matmuls run in bf16 (1 col/cycle vs 4 for fp32).
  - prep(g+1) is hoisted before scores(g) on the PE queue, and all of
    phase 2 for graph g (transpose/attention/outputs) is injected into the
    middle of graph g+1's score stream so the PE never stalls at graph
    boundaries (keeps the HAM clock gate at 2.4GHz too).
  - Input DMAs are issued from the GpSimd queue (25ns/descriptor vs 565ns
    on Sync) with the critical graph-0 slices first; bulk tensors overlap
    on the Sync queue. PE warm-up matmuls run during the DMA wait.
"""

import numpy as np
import ml_dtypes

import concourse.bass as bass
import concourse.tile as tile
from concourse import bacc, mybir
from concourse import bass_utils
from concourse.masks import make_identity

F32 = mybir.dt.float32
BF16 = mybir.dt.bfloat16
AF = mybir.ActivationFunctionType
OP = mybir.AluOpType

B, NA, NB, D = 32, 128, 128, 128
NCORES = 8
G = B // NCORES  # graphs per core
NWARM = 20       # PE warm-up matmuls during the input DMA wait

_CACHE = {}


def _build_program():
    nc = bacc.Bacc(
        "TRN2",
        target_bir_lowering=False,
        debug=False,
        enable_asserts=False,
        num_devices=NCORES,
    )

    # --- DRAM I/O (per core) -------------------------------------------------
    # packed consts: [w1aT | w1bT | w2s | b1(f32 as 2 bf16 cols)] -> one DMA
    wpack_d = nc.dram_tensor("wpack", [D, 2 * D + 160 + 2], BF16,
                             kind="ExternalInput")
    # Group-q stationary for the scores matmul: w2s[h, c] = w2[h]*(c%32==0),
    # c in [0,160); lhsT_q = w2s[:, 32-q : 160-q]. One matmul per q contracts
    # FOUR relu tiles (moving [128, 512]) writing score rows {q,q+32,q+64,q+96};
    # with n = q + 32*j, partition p's own score row lands at free block p//32.
    haTb_d = nc.dram_tensor("haTb", [D, G * NA], BF16, kind="ExternalInput")
    hbTb_d = nc.dram_tensor("hbTb", [D, G * NB], BF16, kind="ExternalInput")
    # [h | -1] bf16, g-blocked along free dim: col g*(D+1)+c
    haEb_d = nc.dram_tensor("haEb", [NA, G * (D + 1)], BF16, kind="ExternalInput")
    hbEb_d = nc.dram_tensor("hbEb", [NB, G * (D + 1)], BF16, kind="ExternalInput")
    # fp32 h for the final mu = h + num*(-1/S) add: col g*D+d
    haF_d = nc.dram_tensor("haF", [NA, G * D], F32, kind="ExternalInput")
    hbF_d = nc.dram_tensor("hbF", [NB, G * D], F32, kind="ExternalInput")
    mua_d = nc.dram_tensor("mu_a", [G * NA, D], F32, kind="ExternalOutput")
    mub_d = nc.dram_tensor("mu_b", [G * NB, D], F32, kind="ExternalOutput")

    mua = mua_d.ap().rearrange("(g n) c -> g n c", g=G)
    mub = mub_d.ap().rearrange("(g n) c -> g n c", g=G)

    with tile.TileContext(nc) as tc:
        with (
            tc.tile_pool(name="data", bufs=1) as data,
            tc.tile_pool(name="gg", bufs=2) as gg,
            tc.tile_pool(name="t", bufs=8) as t_pool,
            tc.tile_pool(name="outs", bufs=4) as out_pool,
            tc.tile_pool(name="prep_ps", bufs=2, space="PSUM") as prep_ps,
            tc.tile_pool(name="sc_ps", bufs=2, space="PSUM") as sc_ps,
            tc.tile_pool(name="ab_ps", bufs=2, space="PSUM") as ab_ps,
        ):
            # --- input DMAs: critical path first, all on the cheap gpsimd queue
            # Graph-0 slices land in their own tiles so the first prep matmul
            # never waits on the bulk (g=1..3) transfers.
            # critical path on the sync queue: wpack, graph-0 transposes
            wpack_sb = data.tile([D, 2 * D + 160 + 2], BF16, tag="wpack")
            nc.sync.dma_start(out=wpack_sb, in_=wpack_d.ap())
            haTb0 = data.tile([D, NA], BF16, tag="haTb0")
            nc.sync.dma_start(out=haTb0, in_=haTb_d.ap()[:, 0:NA])
            hbTb0 = data.tile([D, NB], BF16, tag="hbTb0")
            nc.sync.dma_start(out=hbTb0, in_=hbTb_d.ap()[:, 0:NB])
            w1aT_sb = wpack_sb[:, 0:D]
            w1bT_sb = wpack_sb[:, D:2 * D]
            w2s_sb = wpack_sb[:, 2 * D:2 * D + 160]
            b1_sb = wpack_sb[:, 2 * D + 160:2 * D + 162].bitcast(F32)

            # warm-up fodder + identity first on gpsimd (unblocks PE warm-up)
            warm_sb = data.tile([128, 16], BF16, tag="warm")
            nc.gpsimd.memset(warm_sb, 0.0)
            ident_bf = data.tile([128, 128], BF16, tag="ident")
            make_identity(nc, ident_bf)

            # remaining inputs: per-graph transposes + graph-0 phase-2 slices
            # on gpsimd, bulk phase-2 tensors on sync (all overlap compute)
            E1 = D + 1
            haTbg = [haTb0, None, None, None]
            hbTbg = [hbTb0, None, None, None]
            for g in range(1, G):
                haTbg[g] = data.tile([D, NA], BF16, tag=f"haTb{g}",
                                     name=f"haTb{g}")
                nc.gpsimd.dma_start(out=haTbg[g],
                                    in_=haTb_d.ap()[:, g * NA:(g + 1) * NA])
                hbTbg[g] = data.tile([D, NB], BF16, tag=f"hbTb{g}",
                                     name=f"hbTb{g}")
                nc.gpsimd.dma_start(out=hbTbg[g],
                                    in_=hbTb_d.ap()[:, g * NB:(g + 1) * NB])
            haEb0 = data.tile([NA, E1], BF16, tag="haEb0")
            nc.gpsimd.dma_start(out=haEb0, in_=haEb_d.ap()[:, 0:E1])
            hbEb0 = data.tile([NB, E1], BF16, tag="hbEb0")
            nc.gpsimd.dma_start(out=hbEb0, in_=hbEb_d.ap()[:, 0:E1])
            haF0 = data.tile([NA, D], F32, tag="haF0")
            nc.gpsimd.dma_start(out=haF0, in_=haF_d.ap()[:, 0:D])
            hbF0 = data.tile([NB, D], F32, tag="hbF0")
            nc.gpsimd.dma_start(out=hbF0, in_=hbF_d.ap()[:, 0:D])

            haEbR = data.tile([NA, (G - 1) * E1], BF16, tag="haEbR")
            nc.sync.dma_start(out=haEbR, in_=haEb_d.ap()[:, E1:G * E1])
            hbEbR = data.tile([NB, (G - 1) * E1], BF16, tag="hbEbR")
            nc.sync.dma_start(out=hbEbR, in_=hbEb_d.ap()[:, E1:G * E1])
            haFR = data.tile([NA, (G - 1) * D], F32, tag="haFR")
            nc.sync.dma_start(out=haFR, in_=haF_d.ap()[:, D:G * D])
            hbFR = data.tile([NB, (G - 1) * D], F32, tag="hbFR")
            nc.sync.dma_start(out=hbFR, in_=hbF_d.ap()[:, D:G * D])

            def _slice(g, t0, tR, w):
                return t0 if g == 0 else tR[:, (g - 1) * w:g * w]

            # --- PE warm-up: keep the HAM activity monitor busy while DMAs land
            ps_warm = prep_ps.tile([128, 16], F32, tag="warm", bufs=1)
            for _ in range(NWARM):
                nc.tensor.matmul(ps_warm, lhsT=ident_bf, rhs=warm_sb,
                                 start=True, stop=True)

            # --- build-time greedy load balancer for the relu tiles ----------
            load = {"v": 0.0, "g": 0.0, "a": 0.0}
            COST = {"v": 165.0, "g": 1e9, "a": 295.0}

            def pick():
                e = min(load, key=lambda k: load[k] + COST[k])
                load[e] += COST[e]
                return e

            xa_bf = [None] * G
            xa_f = [None] * G
            xb_bf = [None] * G

            def emit_prep(g):
                # xa_T[h,n] = W1a @ ha^T ; xb'_T[h,m] = W1b @ hb^T + b1 (bf16)
                ps_xa = prep_ps.tile([D, NA], F32, tag="prep")
                nc.tensor.matmul(ps_xa, lhsT=w1aT_sb, rhs=haTbg[g],
                                 start=True, stop=True)
                ps_xb = prep_ps.tile([D, NB], F32, tag="prep")
                nc.tensor.matmul(ps_xb, lhsT=w1bT_sb, rhs=hbTbg[g],
                                 start=True, stop=True)
                return ps_xa, ps_xb

            def emit_conv(g, ps_xa, ps_xb):
                # per-n bias columns (fp32: Bass requires fp32 scalars for add)
                xa_f[g] = gg.tile([D, NA], F32, tag="xaf", name="xaf")
                nc.scalar.copy(out=xa_f[g], in_=ps_xa)
                load["a"] += 200.0
                xb_bf[g] = gg.tile([D, NB], BF16, tag="xb", name="xb")
                nc.vector.tensor_scalar(
                    out=xb_bf[g], in0=ps_xb, scalar1=b1_sb[:, 0:1],
                    scalar2=None, op0=OP.add)
                load["v"] += 260.0

            ps_xa0, ps_xb0 = emit_prep(0)
            emit_conv(0, ps_xa0, ps_xb0)

            e_sb = [None] * G
            et_sb = [None] * G
            ps_sc = [None] * G
            ps_tr = [None] * G
            ps_a = [None] * G
            ps_b = [None] * G
            ra = [None] * G
            rb = [None] * G

            def emit_exp(g):
                # E[n, m] = exp(scores): partition range [32u, 32u+32) holds
                # its own scores at free block u. (softmax is shift-invariant;
                # scores are O(+-10) so no max subtraction.)
                e_sb[g] = gg.tile([NA, NB], BF16, tag="e", name="e")
                for u in range(4):
                    nc.scalar.activation(
                        out=e_sb[g][32 * u:32 * (u + 1), :],
                        in_=ps_sc[g][32 * u:32 * (u + 1), u * NB:(u + 1) * NB],
                        func=AF.Exp)
                load["a"] += 4 * 200.0

            def emit_tr(g):
                ps_tr[g] = ab_ps.tile([NB, NA], BF16, tag="tr", name="tr", bufs=1)
                nc.tensor.transpose(ps_tr[g], e_sb[g], ident_bf)

            def emit_attn_b(g):
                # num_b[m, 0:128], -S_ba[m] at col 128
                ps_b[g] = ab_ps.tile([NB, E1], F32, tag="ab", name="psb")
                nc.tensor.matmul(
                    ps_b[g], lhsT=e_sb[g],
                    rhs=_slice(g, haEb0, haEbR, E1), start=True, stop=True)

            def emit_etcopy(g):
                et_sb[g] = gg.tile([NB, NA], BF16, tag="et", name="et")
                nc.vector.tensor_copy(out=et_sb[g], in_=ps_tr[g])
                load["v"] += 190.0

            def emit_attn_a(g):
                ps_a[g] = ab_ps.tile([NA, E1], F32, tag="ab", name="psa")
                nc.tensor.matmul(
                    ps_a[g], lhsT=et_sb[g],
                    rhs=_slice(g, hbEb0, hbEbR, E1), start=True, stop=True)

            def emit_recip(g):
                ra[g] = out_pool.tile([NA, 1], F32, tag="r", name="ra")
                nc.vector.reciprocal(out=ra[g], in_=ps_a[g][:, D:D + 1])
                rb[g] = out_pool.tile([NB, 1], F32, tag="r", name="rb")
                nc.vector.reciprocal(out=rb[g], in_=ps_b[g][:, D:D + 1])
                load["v"] += 330.0

            def emit_stt(g):
                # mu = h + num * (-1/S)
                outa = out_pool.tile([NA, D], F32, tag="oa")
                nc.vector.scalar_tensor_tensor(
                    out=outa, in0=ps_a[g][:, 0:D], scalar=ra[g][:, 0:1],
                    in1=_slice(g, haF0, haFR, D), op0=OP.mult, op1=OP.add)
                load["v"] += 350.0
                outb = out_pool.tile([NB, D], F32, tag="ob")
                nc.vector.scalar_tensor_tensor(
                    out=outb, in0=ps_b[g][:, 0:D], scalar=rb[g][:, 0:1],
                    in1=_slice(g, hbF0, hbFR, D), op0=OP.mult, op1=OP.add)
                load["v"] += 350.0
                return outa, outb

            def emit_out(g, outa, outb):
                nc.gpsimd.dma_start(out=mua[g], in_=outa)
                nc.gpsimd.dma_start(out=mub[g], in_=outb)

            pend = [None]  # phase-2 closure state for the previous graph
            for g in range(G):
                prev = g - 1
                for q in range(32):
                    # inject previous graph's phase 2 into this score stream
                    if prev >= 0:
                        if q == 5:
                            emit_tr(prev)
                            emit_attn_b(prev)
                        elif q == 7:
                            emit_etcopy(prev)
                        elif q == 9:
                            emit_attn_a(prev)
                        elif q == 11:
                            emit_recip(prev)
                        elif q == 12:
                            pend[0] = emit_stt(prev)
                        elif q == 14:
                            emit_out(prev, *pend[0])
                    if g + 1 < G and q == (6 if g == 0 else 20):
                        ps_xa_n, ps_xb_n = emit_prep(g + 1)
                        emit_conv(g + 1, ps_xa_n, ps_xb_n)

                    # 4 relu tiles t_n = relu(xb' + xa_n), n = q + 32j, then
                    # one accumulating score matmul over the group-q stationary
                    if q == 0:
                        ps_sc[g] = sc_ps.tile([NA, 4 * NB], F32, tag="sc", name="sc")
                    t4 = t_pool.tile([D, 4 * NB], BF16, tag="t")
                    for j in range(4):
                        n = q + 32 * j
                        ts = t4[:, j * NB:(j + 1) * NB]
                        eng = pick()
                        if eng == "a":
                            nc.scalar.activation(
                                out=ts, in_=xb_bf[g], func=AF.Relu,
                                bias=xa_f[g][:, n:n + 1], scale=1.0)
                        elif eng == "v":
                            nc.vector.tensor_scalar(
                                out=ts, in0=xb_bf[g],
                                scalar1=xa_f[g][:, n:n + 1], scalar2=0.0,
                                op0=OP.add, op1=OP.max)
                        else:
                            nc.gpsimd.tensor_scalar(
                                out=ts, in0=xb_bf[g],
                                scalar1=xa_f[g][:, n:n + 1], scalar2=0.0,
                                op0=OP.add, op1=OP.max)
                    nc.tensor.matmul(
                        ps_sc[g], lhsT=w2s_sb[:, 32 - q:160 - q], rhs=t4,
                        start=(q == 0), stop=(q == 31))
                emit_exp(g)

            # drain phase 2 of the last graph
            gl = G - 1
            emit_tr(gl)
            emit_attn_b(gl)
            emit_etcopy(gl)
            emit_attn_a(gl)
            emit_recip(gl)
            oa, ob = emit_stt(gl)
            emit_out(gl, oa, ob)

    nc.compile()
    return nc


def _get_program():
    if "nc" not in _CACHE:
        _CACHE["nc"] = _build_program()
    return _CACHE["nc"]


def _prep_in_maps(h_a, h_b, W1, b1, W2):
    h_a = np.asarray(h_a, dtype=np.float32)
    h_b = np.asarray(h_b, dtype=np.float32)
    W1 = np.asarray(W1, dtype=np.float32)
    b1 = np.asarray(b1, dtype=np.float32)
    W2 = np.asarray(W2, dtype=np.float32)
    bf = ml_dtypes.bfloat16

    # W1a[h, d] = W1[h, d], W1b[h, d] = W1[h, D + d]; lhsT wants [d, h].
    w1aT = np.ascontiguousarray(W1[:, :D].T).astype(bf)
    w1bT = np.ascontiguousarray(W1[:, D:].T).astype(bf)
    w2bf = W2[0].astype(bf).astype(np.float32)
    comb = (np.arange(160) % 32 == 0).astype(np.float32)
    w2s = np.ascontiguousarray(w2bf[:, None] * comb[None, :]).astype(bf)
    # b1 rides along as raw f32 bits in two bf16 columns
    b1bits = np.ascontiguousarray(
        b1.astype(np.float32).reshape(D, 1)).view(bf).reshape(D, 2)
    wpack = np.ascontiguousarray(
        np.concatenate([w1aT, w1bT, w2s, b1bits], axis=1))

    in_maps = []
    for c in range(NCORES):
        ha = h_a[c * G * NA:(c + 1) * G * NA].reshape(G, NA, D)
        hb = h_b[c * G * NB:(c + 1) * G * NB].reshape(G, NB, D)
        neg = np.full((G, NA, 1), -1.0, dtype=np.float32)
        haE = np.concatenate([ha, neg], axis=2)  # [G, NA, D+1]
        hbE = np.concatenate([hb, neg], axis=2)
        in_maps.append({
            "wpack": wpack,
            "haTb": np.ascontiguousarray(
                ha.transpose(2, 0, 1).reshape(D, G * NA)).astype(bf),
            "hbTb": np.ascontiguousarray(
                hb.transpose(2, 0, 1).reshape(D, G * NB)).astype(bf),
            "haEb": np.ascontiguousarray(
                haE.transpose(1, 0, 2).reshape(NA, G * (D + 1))).astype(bf),
            "hbEb": np.ascontiguousarray(
                hbE.transpose(1, 0, 2).reshape(NB, G * (D + 1))).astype(bf),
            "haF": np.ascontiguousarray(ha.transpose(1, 0, 2).reshape(NA, G * D)),
            "hbF": np.ascontiguousarray(hb.transpose(1, 0, 2).reshape(NB, G * D)),
        })
    return in_maps


def run(h_a, h_b, W1, b1, W2, trace=False, **run_kwargs):
    nc = _get_program()
    in_maps = _prep_in_maps(h_a, h_b, W1, b1, W2)
    res = bass_utils.run_bass_kernel_spmd(
        nc, in_maps, core_ids=list(range(NCORES)), trace=trace, **run_kwargs
    )
    mu_a = np.concatenate([r["mu_a"] for r in res.results], axis=0)
    mu_b = np.concatenate([r["mu_b"] for r in res.results], axis=0)
    return (mu_a, mu_b), res


def kernel(h_a, batch_a, h_b, batch_b, W1, b1, W2, b2):
    # batch_a/batch_b encode the (equal-sized, sorted) graph partition that the
    # dense [B, n, D] view already assumes; b2 shifts scores uniformly and
    # cancels in both softmaxes.
    (mu_a, mu_b), _ = run(h_a, h_b, W1, b1, W2, trace=False)
    return mu_a, mu_b


# revision 22
# speedup vs baseline: 4.7805x; 1.0469x over previous
"""Cross-graph attention kernel for Trainium2 (8 NeuronCores, SPMD data-parallel over B).

Problem (B=32 graphs, NA=NB=128 nodes, D=128):
    xa = ha @ W1a.T ; xb = hb @ W1b.T                      (per graph)
    scores[n,m] = sum_h relu(xa[n,h] + xb[m,h] + b1[h]) * w2[h]  (+ b2, which
                  cancels in both softmaxes and is dropped)
    mu_a = ha - softmax_m(scores) @ hb
    mu_b = hb - softmax_n(scores).T @ ha

Sharding: data-parallel over B across 8 cores (4 graphs/core), sim_net params
replicated. All pairwise intermediates stay in SBUF/PSUM.

v2 design notes (from ntff trace analysis of the v1 baseline, 87us):
  - Phase 1 (relu tiles feeding the score matmuls) was producer-bound:
    DVE tensor_scalar 162ns + ACT relu 293ns per [128,128] tile -> ~410ns
    per 4-tile group vs 215ns warm PE matmul. Fix: add GpSimd as a third
    producer (build-time greedy load balancing across DVE/GpSimd/ACT).
  - b1 is folded into xb once per graph (xb' = xb + b1), so the per-n bias
    column is just xa[:, n] - no separate xa+b1 tensor needed.
  - prep matmuls run in bf16 (1 col/cycle vs 4 for fp32).
  - prep(g+1) is hoisted before scores(g) on the PE queue, and all of
    phase 2 for graph g (transpose/attention/outputs) is injected into the
    middle of graph g+1's score stream so the PE never stalls at graph
    boundaries (keeps the HAM clock gate at 2.4GHz too).
  - Input DMAs are issued from the GpSimd queue (25ns/descriptor vs 565ns
    on Sync) with the critical graph-0 slices first; bulk tensors overlap
    on the Sync queue. PE warm-up matmuls run during the DMA wait.
"""

import numpy as np
import ml_dtypes

import concourse.bass as bass
import concourse.tile as tile
from concourse import bacc, mybir
from concourse import bass_utils
from concourse.masks import make_identity

F32 = mybir.dt.float32
BF16 = mybir.dt.bfloat16
AF = mybir.ActivationFunctionType
OP = mybir.AluOpType

B, NA, NB, D = 32, 128, 128, 128
NCORES = 8
G = B // NCORES  # graphs per core
NWARM = 20       # PE warm-up matmuls during the input DMA wait

_CACHE = {}


def _build_program():
    nc = bacc.Bacc(
        "TRN2",
        target_bir_lowering=False,
        debug=False,
        enable_asserts=False,
        num_devices=NCORES,
    )

    # --- DRAM I/O (per core) -------------------------------------------------
    # packed consts: [w1aT | w1bT | w2s | b1(f32 as 2 bf16 cols)] -> one DMA
    wpack_d = nc.dram_tensor("wpack", [D, 2 * D + 160 + 2], BF16,
                             kind="ExternalInput")
    # Group-q stationary for the scores matmul: w2s[h, c] = w2[h]*(c%32==0),
    # c in [0,160); lhsT_q = w2s[:, 32-q : 160-q]. One matmul per q contracts
    # FOUR relu tiles (moving [128, 512]) writing score rows {q,q+32,q+64,q+96};
    # with n = q + 32*j, partition p's own score row lands at free block p//32.
    haTb_d = nc.dram_tensor("haTb", [D, G * NA], BF16, kind="ExternalInput")
    hbTb_d = nc.dram_tensor("hbTb", [D, G * NB], BF16, kind="ExternalInput")
    # [h | -1] bf16, g-blocked along free dim: col g*(D+1)+c
    haEb_d = nc.dram_tensor("haEb", [NA, G * (D + 1)], BF16, kind="ExternalInput")
    hbEb_d = nc.dram_tensor("hbEb", [NB, G * (D + 1)], BF16, kind="ExternalInput")
    # fp32 h for the final mu = h + num*(-1/S) add: col g*D+d
    haF_d = nc.dram_tensor("haF", [NA, G * D], F32, kind="ExternalInput")
    hbF_d = nc.dram_tensor("hbF", [NB, G * D], F32, kind="ExternalInput")
    mua_d = nc.dram_tensor("mu_a", [G * NA, D], F32, kind="ExternalOutput")
    mub_d = nc.dram_tensor("mu_b", [G * NB, D], F32, kind="ExternalOutput")

    mua = mua_d.ap().rearrange("(g n) c -> g n c", g=G)
    mub = mub_d.ap().rearrange("(g n) c -> g n c", g=G)

    with tile.TileContext(nc) as tc:
        with (
            tc.tile_pool(name="data", bufs=1) as data,
            tc.tile_pool(name="gg", bufs=2) as gg,
            tc.tile_pool(name="t", bufs=8) as t_pool,
            tc.tile_pool(name="outs", bufs=4) as out_pool,
            tc.tile_pool(name="prep_ps", bufs=2, space="PSUM") as prep_ps,
            tc.tile_pool(name="sc_ps", bufs=2, space="PSUM") as sc_ps,
            tc.tile_pool(name="ab_ps", bufs=2, space="PSUM") as ab_ps,
            tc.tile_pool(name="t4_ps", bufs=2, space="PSUM") as t4_ps,
        ):
            # --- input DMAs: critical path first, all on the cheap gpsimd queue
            # Graph-0 slices land in their own tiles so the first prep matmul
            # never waits on the bulk (g=1..3) transfers.
            # critical path on the sync queue: wpack, graph-0 transposes
            wpack_sb = data.tile([D, 2 * D + 160 + 2], BF16, tag="wpack")
            nc.sync.dma_start(out=wpack_sb, in_=wpack_d.ap())
            haTb0 = data.tile([D, NA], BF16, tag="haTb0")
            nc.sync.dma_start(out=haTb0, in_=haTb_d.ap()[:, 0:NA])
            hbTb0 = data.tile([D, NB], BF16, tag="hbTb0")
            nc.sync.dma_start(out=hbTb0, in_=hbTb_d.ap()[:, 0:NB])
            w1aT_sb = wpack_sb[:, 0:D]
            w1bT_sb = wpack_sb[:, D:2 * D]
            w2s_sb = wpack_sb[:, 2 * D:2 * D + 160]
            b1_sb = wpack_sb[:, 2 * D + 160:2 * D + 162].bitcast(F32)

            # warm-up fodder + identity first on gpsimd (unblocks PE warm-up)
            warm_sb = data.tile([128, 16], BF16, tag="warm")
            nc.gpsimd.memset(warm_sb, 0.0)
            ident_bf = data.tile([128, 128], BF16, tag="ident")
            make_identity(nc, ident_bf)
            # preload the ACT spline tables (~1.3us) off the critical path
            act_pre = data.tile([128, 1], F32, tag="act_pre")
            nc.scalar.activation(out=act_pre, in_=warm_sb[:, 0:1],
                                 func=AF.Relu)

            # remaining inputs: per-graph transposes + graph-0 phase-2 slices
            # on gpsimd, bulk phase-2 tensors on sync (all overlap compute)
            E1 = D + 1
            haTbg = [haTb0, None, None, None]
            hbTbg = [hbTb0, None, None, None]
            for g in range(1, G):
                haTbg[g] = data.tile([D, NA], BF16, tag=f"haTb{g}",
                                     name=f"haTb{g}")
                nc.gpsimd.dma_start(out=haTbg[g],
                                    in_=haTb_d.ap()[:, g * NA:(g + 1) * NA])
                hbTbg[g] = data.tile([D, NB], BF16, tag=f"hbTb{g}",
                                     name=f"hbTb{g}")
                nc.gpsimd.dma_start(out=hbTbg[g],
                                    in_=hbTb_d.ap()[:, g * NB:(g + 1) * NB])
            haEb0 = data.tile([NA, E1], BF16, tag="haEb0")
            nc.gpsimd.dma_start(out=haEb0, in_=haEb_d.ap()[:, 0:E1])
            hbEb0 = data.tile([NB, E1], BF16, tag="hbEb0")
            nc.gpsimd.dma_start(out=hbEb0, in_=hbEb_d.ap()[:, 0:E1])
            haF0 = data.tile([NA, D], F32, tag="haF0")
            nc.gpsimd.dma_start(out=haF0, in_=haF_d.ap()[:, 0:D])
            hbF0 = data.tile([NB, D], F32, tag="hbF0")
            nc.gpsimd.dma_start(out=hbF0, in_=hbF_d.ap()[:, 0:D])

            haEbR = data.tile([NA, (G - 1) * E1], BF16, tag="haEbR")
            nc.sync.dma_start(out=haEbR, in_=haEb_d.ap()[:, E1:G * E1])
            hbEbR = data.tile([NB, (G - 1) * E1], BF16, tag="hbEbR")
            nc.sync.dma_start(out=hbEbR, in_=hbEb_d.ap()[:, E1:G * E1])
            haFR = data.tile([NA, (G - 1) * D], F32, tag="haFR")
            nc.sync.dma_start(out=haFR, in_=haF_d.ap()[:, D:G * D])
            hbFR = data.tile([NB, (G - 1) * D], F32, tag="hbFR")
            nc.sync.dma_start(out=hbFR, in_=hbF_d.ap()[:, D:G * D])

            def _slice(g, t0, tR, w):
                return t0 if g == 0 else tR[:, (g - 1) * w:g * w]

            # --- PE warm-up: keep the HAM activity monitor busy while DMAs land
            ps_warm = t4_ps.tile([128, 16], F32, tag="t4", name="ps_warm")
            for _ in range(NWARM):
                nc.tensor.matmul(ps_warm, lhsT=ident_bf, rhs=warm_sb,
                                 start=True, stop=True)

            # --- build-time greedy load balancer for the relu tiles ----------
            load = {"v": 0.0, "g": 0.0, "a": 0.0}
            COST = {"v": 165.0, "g": 1e9, "a": 295.0}

            def pick():
                e = min(load, key=lambda k: load[k] + COST[k])
                load[e] += COST[e]
                return e

            xa_bf = [None] * G
            xa_f = [None] * G
            xb_bf = [None] * G

            def emit_prep(g):
                # xa_T[h,n] = W1a @ ha^T ; xb'_T[h,m] = W1b @ hb^T + b1 (bf16)
                ps_xa = prep_ps.tile([D, NA], F32, tag="prep")
                nc.tensor.matmul(ps_xa, lhsT=w1aT_sb, rhs=haTbg[g],
                                 start=True, stop=True)
                ps_xb = prep_ps.tile([D, NB], F32, tag="prep")
                nc.tensor.matmul(ps_xb, lhsT=w1bT_sb, rhs=hbTbg[g],
                                 start=True, stop=True)
                return ps_xa, ps_xb

            def emit_conv(g, ps_xa, ps_xb):
                # per-n bias columns (fp32: Bass requires fp32 scalars for add)
                xa_f[g] = gg.tile([D, NA], F32, tag="xaf", name="xaf")
                nc.scalar.copy(out=xa_f[g], in_=ps_xa)
                load["a"] += 200.0
                xa_bf[g] = gg.tile([D, NA], BF16, tag="xa", name="xa")
                nc.vector.tensor_copy(out=xa_bf[g], in_=ps_xa)
                load["v"] += 260.0
                xb_bf[g] = gg.tile([D, NB], BF16, tag="xb", name="xb")
                nc.vector.tensor_scalar(
                    out=xb_bf[g], in0=ps_xb, scalar1=b1_sb[:, 0:1],
                    scalar2=None, op0=OP.add)
                load["v"] += 260.0

            ps_xa0, ps_xb0 = emit_prep(0)
            emit_conv(0, ps_xa0, ps_xb0)

            e_sb = [None] * G
            et_sb = [None] * G
            ps_sc = [None] * G
            ps_tr = [None] * G
            ps_a = [None] * G
            ps_b = [None] * G
            ra = [None] * G
            rb = [None] * G

            def emit_exp(g):
                # E[n, m] = exp(scores): partition range [32u, 32u+32) holds
                # its own scores at free block u. (softmax is shift-invariant;
                # scores are O(+-10) so no max subtraction.)
                e_sb[g] = gg.tile([NA, NB], BF16, tag="e", name="e")
                for u in range(4):
                    nc.scalar.activation(
                        out=e_sb[g][32 * u:32 * (u + 1), :],
                        in_=ps_sc[g][32 * u:32 * (u + 1), u * NB:(u + 1) * NB],
                        func=AF.Exp)
                load["a"] += 4 * 200.0

            def emit_tr(g):
                ps_tr[g] = ab_ps.tile([NB, NA], BF16, tag="ab", name="tr")
                nc.tensor.transpose(ps_tr[g], e_sb[g], ident_bf)

            def emit_attn_b(g):
                # num_b[m, 0:128], -S_ba[m] at col 128
                ps_b[g] = ab_ps.tile([NB, E1], F32, tag="ab", name="psb")
                nc.tensor.matmul(
                    ps_b[g], lhsT=e_sb[g],
                    rhs=_slice(g, haEb0, haEbR, E1), start=True, stop=True)

            def emit_etcopy(g):
                et_sb[g] = gg.tile([NB, NA], BF16, tag="et", name="et")
                nc.vector.tensor_copy(out=et_sb[g], in_=ps_tr[g])
                load["v"] += 190.0

            def emit_attn_a(g):
                ps_a[g] = ab_ps.tile([NA, E1], F32, tag="ab", name="psa")
                nc.tensor.matmul(
                    ps_a[g], lhsT=et_sb[g],
                    rhs=_slice(g, hbEb0, hbEbR, E1), start=True, stop=True)

            def emit_recip(g):
                ra[g] = out_pool.tile([NA, 1], F32, tag="r", name="ra")
                nc.vector.reciprocal(out=ra[g], in_=ps_a[g][:, D:D + 1])
                rb[g] = out_pool.tile([NB, 1], F32, tag="r", name="rb")
                nc.vector.reciprocal(out=rb[g], in_=ps_b[g][:, D:D + 1])
                load["v"] += 330.0

            def emit_stt(g):
                # mu = h + num * (-1/S)
                outa = out_pool.tile([NA, D], F32, tag="oa")
                nc.vector.scalar_tensor_tensor(
                    out=outa, in0=ps_a[g][:, 0:D], scalar=ra[g][:, 0:1],
                    in1=_slice(g, haF0, haFR, D), op0=OP.mult, op1=OP.add)
                load["v"] += 350.0
                outb = out_pool.tile([NB, D], F32, tag="ob")
                nc.vector.scalar_tensor_tensor(
                    out=outb, in0=ps_b[g][:, 0:D], scalar=rb[g][:, 0:1],
                    in1=_slice(g, hbF0, hbFR, D), op0=OP.mult, op1=OP.add)
                load["v"] += 350.0
                return outa, outb

            def emit_out(g, outa, outb):
                if g == G - 1:
                    nc.sync.dma_start(out=mua[g][0:64, :], in_=outa[0:64, :])
                    nc.gpsimd.dma_start(out=mua[g][64:128, :],
                                        in_=outa[64:128, :])
                    nc.sync.dma_start(out=mub[g][0:64, :], in_=outb[0:64, :])
                    nc.gpsimd.dma_start(out=mub[g][64:128, :],
                                        in_=outb[64:128, :])
                else:
                    nc.gpsimd.dma_start(out=mua[g], in_=outa)
                    nc.gpsimd.dma_start(out=mub[g], in_=outb)

            # route-B q's: PE builds relu args in PSUM (identity stationary,
            # broadcast moving APs), then ONE [128,512] relu on ACT/DVE
            BSET = frozenset(q for q in range(32) if q % 3 == 1)
            pend_arg = {}

            def emit_argmm(g, q):
                pt = t4_ps.tile([D, 4 * NB], F32, tag="t4", name="t4ps")
                xa4 = xa_bf[g][:, q:q + 97:32]
                mova = xa4.rearrange("p (n o) -> p n o", o=1).broadcast_to(
                    [D, 4, NB])
                nc.tensor.matmul(pt, lhsT=ident_bf, rhs=mova,
                                 start=True, stop=False)
                movb = xb_bf[g].rearrange("p (o m) -> p o m", o=1).broadcast_to(
                    [D, 4, NB])
                nc.tensor.matmul(pt, lhsT=ident_bf, rhs=movb,
                                 start=False, stop=True)
                pend_arg[q] = pt

            pend = [None]  # phase-2 closure state for the previous graph
            for g in range(G):
                prev = g - 1
                for q in range(32):
                    # inject previous graph's phase 2 into this score stream
                    if prev >= 0:
                        if q == 5:
                            emit_tr(prev)
                            emit_attn_b(prev)
                        elif q == 7:
                            emit_etcopy(prev)
                        elif q == 9:
                            emit_attn_a(prev)
                        elif q == 11:
                            emit_recip(prev)
                        elif q == 12:
                            pend[0] = emit_stt(prev)
                        elif q == 14:
                            emit_out(prev, *pend[0])
                    if g + 1 < G and q == (6 if g == 0 else 20):
                        ps_xa_n, ps_xb_n = emit_prep(g + 1)
                        emit_conv(g + 1, ps_xa_n, ps_xb_n)

                    # 4 relu tiles t_n = relu(xb' + xa_n), n = q + 32j, then
                    # one accumulating score matmul over the group-q stationary
                    if q == 0:
                        ps_sc[g] = sc_ps.tile([NA, 4 * NB], F32, tag="sc", name="sc")
                    # pipeline: next route-B q's arg matmuls go ahead of this
                    # q's score matmul so the PE never waits on the big relu
                    if q + 1 in BSET:
                        emit_argmm(g, q + 1)
                    t4 = t_pool.tile([D, 4 * NB], BF16, tag="t")
                    if q in BSET:
                        pt = pend_arg.pop(q)
                        if load["a"] + 560.0 < load["v"] + 625.0:
                            nc.scalar.activation(out=t4, in_=pt, func=AF.Relu)
                            load["a"] += 560.0
                        else:
                            nc.vector.tensor_scalar(
                                out=t4, in0=pt, scalar1=0.0, scalar2=None,
                                op0=OP.max)
                            load["v"] += 625.0
                    else:
                        for j in range(4):
                            n = q + 32 * j
                            ts = t4[:, j * NB:(j + 1) * NB]
                            eng = pick()
                            if eng == "a":
                                nc.scalar.activation(
                                    out=ts, in_=xb_bf[g], func=AF.Relu,
                                    bias=xa_f[g][:, n:n + 1], scale=1.0)
                            else:
                                nc.vector.tensor_scalar(
                                    out=ts, in0=xb_bf[g],
                                    scalar1=xa_f[g][:, n:n + 1], scalar2=0.0,
                                    op0=OP.add, op1=OP.max)
                    nc.tensor.matmul(
                        ps_sc[g], lhsT=w2s_sb[:, 32 - q:160 - q], rhs=t4,
                        start=(q == 0), stop=(q == 31))
                emit_exp(g)

            # drain phase 2 of the last graph
            gl = G - 1
            emit_tr(gl)
            emit_attn_b(gl)
            emit_etcopy(gl)
            emit_attn_a(gl)
            emit_recip(gl)
            oa, ob = emit_stt(gl)
            emit_out(gl, oa, ob)

    nc.compile()
    return nc


def _get_program():
    if "nc" not in _CACHE:
        _CACHE["nc"] = _build_program()
    return _CACHE["nc"]


def _prep_in_maps(h_a, h_b, W1, b1, W2):
    h_a = np.asarray(h_a, dtype=np.float32)
    h_b = np.asarray(h_b, dtype=np.float32)
    W1 = np.asarray(W1, dtype=np.float32)
    b1 = np.asarray(b1, dtype=np.float32)
    W2 = np.asarray(W2, dtype=np.float32)
    bf = ml_dtypes.bfloat16

    # W1a[h, d] = W1[h, d], W1b[h, d] = W1[h, D + d]; lhsT wants [d, h].
    w1aT = np.ascontiguousarray(W1[:, :D].T).astype(bf)
    w1bT = np.ascontiguousarray(W1[:, D:].T).astype(bf)
    w2bf = W2[0].astype(bf).astype(np.float32)
    comb = (np.arange(160) % 32 == 0).astype(np.float32)
    w2s = np.ascontiguousarray(w2bf[:, None] * comb[None, :]).astype(bf)
    # b1 rides along as raw f32 bits in two bf16 columns
    b1bits = np.ascontiguousarray(
        b1.astype(np.float32).reshape(D, 1)).view(bf).reshape(D, 2)
    wpack = np.ascontiguousarray(
        np.concatenate([w1aT, w1bT, w2s, b1bits], axis=1))

    in_maps = []
    for c in range(NCORES):
        ha = h_a[c * G * NA:(c + 1) * G * NA].reshape(G, NA, D)
        hb = h_b[c * G * NB:(c + 1) * G * NB].reshape(G, NB, D)
        neg = np.full((G, NA, 1), -1.0, dtype=np.float32)
        haE = np.concatenate([ha, neg], axis=2)  # [G, NA, D+1]
        hbE = np.concatenate([hb, neg], axis=2)
        in_maps.append({
            "wpack": wpack,
            "haTb": np.ascontiguousarray(
                ha.transpose(2, 0, 1).reshape(D, G * NA)).astype(bf),
            "hbTb": np.ascontiguousarray(
                hb.transpose(2, 0, 1).reshape(D, G * NB)).astype(bf),
            "haEb": np.ascontiguousarray(
                haE.transpose(1, 0, 2).reshape(NA, G * (D + 1))).astype(bf),
            "hbEb": np.ascontiguousarray(
                hbE.transpose(1, 0, 2).reshape(NB, G * (D + 1))).astype(bf),
            "haF": np.ascontiguousarray(ha.transpose(1, 0, 2).reshape(NA, G * D)),
            "hbF": np.ascontiguousarray(hb.transpose(1, 0, 2).reshape(NB, G * D)),
        })
    return in_maps


def run(h_a, h_b, W1, b1, W2, trace=False, **run_kwargs):
    nc = _get_program()
    in_maps = _prep_in_maps(h_a, h_b, W1, b1, W2)
    res = bass_utils.run_bass_kernel_spmd(
        nc, in_maps, core_ids=list(range(NCORES)), trace=trace, **run_kwargs
    )
    mu_a = np.concatenate([r["mu_a"] for r in res.results], axis=0)
    mu_b = np.concatenate([r["mu_b"] for r in res.results], axis=0)
    return (mu_a, mu_b), res


def kernel(h_a, batch_a, h_b, batch_b, W1, b1, W2, b2):
    # batch_a/batch_b encode the (equal-sized, sorted) graph partition that the
    # dense [B, n, D] view already assumes; b2 shifts scores uniformly and
    # cancels in both softmaxes.
    (mu_a, mu_b), _ = run(h_a, h_b, W1, b1, W2, trace=False)
    return mu_a, mu_b
